# revision 1
# baseline (speedup 1.0000x reference)
"""RNN-T joint network kernel for Trainium2 (Bass/Tile), 8-core data-parallel.

Problem: out[b,t,u,:] = tanh(enc[b,t]@W_enc + b_enc + dec[b,u]@W_dec + b_dec) @ W_out + b_out
Shapes: B=8, T=256, U=64, D=512, J=640, V=1024 (all fp32).

Sharding: data-parallel over batch B across the 8 NeuronCores (1 batch element
per core). Per core the dominant work is the joint matmul (B,T,U,J)x(J,V):
20.5 GFLOP -> ~273us at 1 col/cycle (bf16 matmul), vs ~180us HBM write for
the 64MB output slice -> PE-bound "ridge" regime.

Per-core plan (all J-major layouts so J is the matmul contraction partition dim):
  setup:  host passes encT (512,256) / decT (512,64) pre-transposed;
          enc_projT[j,t] = (W_enc^T @ encT) (no bias), dec_projT[j,u] = W_dec^T@decT
          + (b_enc+b_dec) folded in via ACT bias on the PSUM->SBUF copy.
  main:   for each u: hT[j,t] = tanh(enc_projT[j,t] + dec_projT[j,u]) via ACT
          (bias = per-partition dec column, broadcast along free dim t);
          joint matmul out(t,1024) = hT^T @ W_out in bf16 (1 cyc/col, fp32 PSUM);
          DVE adds broadcast b_out while draining PSUM->SBUF; DMA out in
          2MB strided chunks (u-blocks of 4).
"""

import numpy as np
from contextlib import ExitStack

from concourse import bacc, bass, tile
from concourse.bass import mybir
from concourse.bass_utils import run_bass_kernel_spmd

F32 = mybir.dt.float32
BF16 = mybir.dt.bfloat16
ACT_F = mybir.ActivationFunctionType

B, T, U = 8, 256, 64
D, J, V = 512, 640, 1024
NJC = J // 128   # 5 contraction chunks of the joint matmul
NDC = D // 128   # 4 contraction chunks of the projections
UB = 4           # u-block staged per output DMA
NVB = V // 512   # 2 psum banks per joint output tile


def build_program() -> bass.Bass:
    nc = bacc.Bacc("TRN2", target_bir_lowering=False, debug=False)

    encT_d = nc.declare_dram_parameter("encT", [D, T], F32, isOutput=False)
    decT_d = nc.declare_dram_parameter("decT", [D, U], F32, isOutput=False)
    w_enc = nc.declare_dram_parameter("w_enc", [D, J], F32, isOutput=False)
    w_dec = nc.declare_dram_parameter("w_dec", [D, J], F32, isOutput=False)
    bb = nc.declare_dram_parameter("bb", [J], F32, isOutput=False)  # b_enc+b_dec
    w_out = nc.declare_dram_parameter("w_out", [J, V], BF16, isOutput=False)
    b_out = nc.declare_dram_parameter("b_out", [V], F32, isOutput=False)
    out = nc.declare_dram_parameter("out", [T, U, V], F32, isOutput=True)

    with tile.TileContext(nc) as tc, ExitStack() as ctx:
        const = ctx.enter_context(tc.tile_pool(name="const", bufs=1))

        # --- resident constants -------------------------------------------
        w_out_sb = []
        for jc in range(NJC):
            t = const.tile([128, V], BF16, tag=f"wout{jc}")
            nc.sync.dma_start(out=t[:], in_=w_out[jc * 128 : (jc + 1) * 128, :])
            w_out_sb.append(t)

        bias_rep = const.tile([128, V], F32)
        nc.gpsimd.dma_start(
            out=bias_rep[:],
            in_=b_out[:].unsqueeze(0).broadcast_to((128, V)),
        )

        bbt = []
        for jc in range(NJC):
            t = const.tile([128, 1], F32, tag=f"bb{jc}")
            nc.sync.dma_start(
                out=t[:], in_=bb[jc * 128 : (jc + 1) * 128].unsqueeze(1)
            )
            bbt.append(t)

        enc_projT = [const.tile([128, T], F32, tag=f"ep{jc}", name=f"ep{jc}") for jc in range(NJC)]
        dec_projT = [const.tile([128, U], F32, tag=f"dp{jc}", name=f"dp{jc}") for jc in range(NJC)]

        # --- setup: transposes + input projections ------------------------
        with (
            tc.tile_pool(name="setup", bufs=2) as setup,
            tc.tile_pool(name="setup_ps", bufs=2, space="PSUM") as setup_ps,
            tc.tile_pool(name="setup_keep", bufs=1) as setup_keep,
        ):
            w_enc_sb = []
            w_dec_sb = []
            for dc in range(NDC):
                t = setup_keep.tile([128, J], F32, tag=f"we{dc}")
                nc.sync.dma_start(out=t[:], in_=w_enc[dc * 128 : (dc + 1) * 128, :])
                w_enc_sb.append(t)
                t = setup_keep.tile([128, J], F32, tag=f"wd{dc}")
                nc.sync.dma_start(out=t[:], in_=w_dec[dc * 128 : (dc + 1) * 128, :])
                w_dec_sb.append(t)

            encT = [setup_keep.tile([128, T], F32, tag=f"encT{dc}", name=f"encT{dc}") for dc in range(NDC)]
            decT = [setup_keep.tile([128, U], F32, tag=f"decT{dc}", name=f"decT{dc}") for dc in range(NDC)]
            for dc in range(NDC):
                nc.sync.dma_start(out=encT[dc][:], in_=encT_d[dc * 128 : (dc + 1) * 128, :])
                nc.sync.dma_start(out=decT[dc][:], in_=decT_d[dc * 128 : (dc + 1) * 128, :])

            for jc in range(NJC):
                ps = setup_ps.tile([128, T], F32, tag="proj")
                for dc in range(NDC):
                    nc.tensor.matmul(
                        ps[:],
                        w_enc_sb[dc][:, jc * 128 : (jc + 1) * 128],
                        encT[dc][:],
                        start=(dc == 0),
                        stop=(dc == NDC - 1),
                    )
                nc.scalar.copy(enc_projT[jc][:], ps[:])

                ps = setup_ps.tile([128, U], F32, tag="proj")
                for dc in range(NDC):
                    nc.tensor.matmul(
                        ps[:],
                        w_dec_sb[dc][:, jc * 128 : (jc + 1) * 128],
                        decT[dc][:],
                        start=(dc == 0),
                        stop=(dc == NDC - 1),
                    )
                # fold b_enc+b_dec into dec_projT during the PSUM->SBUF copy
                nc.scalar.activation(
                    dec_projT[jc][:], ps[:], ACT_F.Identity, bias=bbt[jc][:], scale=1.0
                )

        # --- main loop over u-blocks --------------------------------------
        h_pool = ctx.enter_context(tc.tile_pool(name="h", bufs=3))
        st_pool = ctx.enter_context(tc.tile_pool(name="stage", bufs=2))
        mm_ps = ctx.enter_context(tc.tile_pool(name="mm_ps", bufs=3, space="PSUM"))

        for ub in range(U // UB):
            u0 = ub * UB
            hT = [
                [
                    h_pool.tile([128, T], BF16, tag=f"h{jc}_{uu}", name=f"h{jc}_{uu}")
                    for uu in range(UB)
                ]
                for jc in range(NJC)
            ]
            for jc in range(NJC):
                for uu in range(UB):
                    nc.scalar.activation(
                        hT[jc][uu][:],
                        enc_projT[jc][:],
                        ACT_F.Tanh,
                        bias=dec_projT[jc][:, u0 + uu : u0 + uu + 1],
                        scale=1.0,
                    )
            for tt in range(T // 128):
                stage = st_pool.tile([128, UB, V], F32, tag=f"st{tt}")
                for uu in range(UB):
                    ps = mm_ps.tile([128, V], F32, tag="mm")
                    for vv in range(NVB):
                        for jc in range(NJC):
                            nc.tensor.matmul(
                                ps[:, vv * 512 : (vv + 1) * 512],
                                hT[jc][uu][:, tt * 128 : (tt + 1) * 128],
                                w_out_sb[jc][:, vv * 512 : (vv + 1) * 512],
                                start=(jc == 0),
                                stop=(jc == NJC - 1),
                            )
                    # drain PSUM -> SBUF while adding the broadcast b_out
                    nc.vector.tensor_add(stage[:, uu, :], ps[:], bias_rep[:])
                nc.sync.dma_start(
                    out=out[tt * 128 : (tt + 1) * 128, u0 : u0 + UB, :],
                    in_=stage[:],
                )

    nc.finalize()
    return nc


_PROGRAM = None


def kernel(enc_out, dec_out, W_enc, b_enc, W_dec, b_dec, W_out, b_out):
    global _PROGRAM
    if _PROGRAM is None:
        _PROGRAM = build_program()

    bb = (np.asarray(b_enc, np.float32) + np.asarray(b_dec, np.float32))
    import ml_dtypes
    w_out_bf16 = np.asarray(W_out, np.float32).astype(ml_dtypes.bfloat16)
    in_maps = []
    for b in range(B):
        in_maps.append(
            {
                "encT": np.ascontiguousarray(np.asarray(enc_out, np.float32)[b, :, 0, :].T),
                "decT": np.ascontiguousarray(np.asarray(dec_out, np.float32)[b, 0, :, :].T),
                "w_enc": np.asarray(W_enc, np.float32),
                "w_dec": np.asarray(W_dec, np.float32),
                "bb": bb,
                "w_out": w_out_bf16,
                "b_out": np.asarray(b_out, np.float32),
            }
        )
    res = run_bass_kernel_spmd(_PROGRAM, in_maps, list(range(B)))
    return np.stack([res.results[b]["out"] for b in range(B)], axis=0)



# revision 2
# speedup vs baseline: 1.3027x; 1.3027x over previous
"""RNN-T joint network kernel for Trainium2 (Bass/Tile), 8-core data-parallel.

Problem: out[b,t,u,:] = tanh(enc[b,t]@W_enc + b_enc + dec[b,u]@W_dec + b_dec) @ W_out + b_out
Shapes: B=8, T=256, U=64, D=512, J=640, V=1024 (all fp32).

Sharding: data-parallel over batch B across the 8 NeuronCores (1 batch element
per core). Per core the dominant work is the joint matmul (T,U,J)x(J,V):
1280 bf16 matmuls of N=512 -> ~332us at the observed 2.0 GHz PE clock; the
64MB fp32 output DMA (~187us) and the tanh/drain engines fit underneath.

Per-core plan (all J-major layouts so J is the matmul contraction partition dim):
  host:   inputs pre-transposed AND pre-packed per 128-row chunk into single
          [128, n_chunks*width] arrays so every constant loads with ONE
          contiguous DMA descriptor (setup head is DMA-issue-rate bound).
          Projection operands in bf16 (single-pass matmuls; fp32 would lower
          to LOW/HIGH double-pass and double the setup time).
  setup:  enc_projT[j,t] = W_enc^T @ encT (bf16 mms, fp32 PSUM), dec_projT
          likewise with (b_enc+b_dec) folded in via ACT bias on the drain.
  main:   for each u: hT[j,t] = tanh(enc_projT[j,t] + dec_projT[j,u]) via ACT
          (bias = per-partition dec column); joint matmul out(t,512)x2 =
          hT^T @ W_out in bf16 (fp32 PSUM, 4 psum tiles = all 8 banks);
          DVE adds broadcast b_out while draining PSUM->SBUF; DMA out per
          (t-tile, u) as a [128, 4KB-contiguous] transfer.
"""

import numpy as np
from contextlib import ExitStack

from concourse import bacc, bass, tile
from concourse.bass import mybir
from concourse.bass_utils import run_bass_kernel_spmd

F32 = mybir.dt.float32
BF16 = mybir.dt.bfloat16
ACT_F = mybir.ActivationFunctionType

B, T, U = 8, 256, 64
D, J, V = 512, 640, 1024
NJC = J // 128   # 5 contraction chunks of the joint matmul
NDC = D // 128   # 4 contraction chunks of the projections
UB = 4           # u-block whose tanh tiles are generated together
NVB = V // 512   # 2 psum banks per joint output tile


def build_program() -> bass.Bass:
    nc = bacc.Bacc("TRN2", target_bir_lowering=False, debug=False)

    # packed layouts: pk[p, c*W + x] = orig[c*128 + p, x]
    encT_d = nc.declare_dram_parameter("encT", [128, NDC * T], BF16, isOutput=False)
    decT_d = nc.declare_dram_parameter("decT", [128, NDC * U], BF16, isOutput=False)
    w_enc_d = nc.declare_dram_parameter("w_enc", [128, NDC * J], BF16, isOutput=False)
    w_dec_d = nc.declare_dram_parameter("w_dec", [128, NDC * J], BF16, isOutput=False)
    bb_d = nc.declare_dram_parameter("bb", [128, NJC], F32, isOutput=False)  # b_enc+b_dec
    w_out_d = nc.declare_dram_parameter("w_out", [128, NJC * V], BF16, isOutput=False)
    b_out_d = nc.declare_dram_parameter("b_out", [V], F32, isOutput=False)
    out = nc.declare_dram_parameter("out", [T, U, V], F32, isOutput=True)

    with tile.TileContext(nc) as tc, ExitStack() as ctx:
        const = ctx.enter_context(tc.tile_pool(name="const", bufs=1))

        # --- resident constants, issued in dependency-priority order --------
        bbt = const.tile([128, NJC], F32)
        nc.sync.dma_start(out=bbt[:], in_=bb_d[:])
        decT = const.tile([128, NDC * U], BF16)
        nc.sync.dma_start(out=decT[:], in_=decT_d[:])
        encT = const.tile([128, NDC * T], BF16)
        nc.sync.dma_start(out=encT[:], in_=encT_d[:])
        w_dec_sb = const.tile([128, NDC * J], BF16)
        nc.sync.dma_start(out=w_dec_sb[:], in_=w_dec_d[:])
        w_enc_sb = const.tile([128, NDC * J], BF16)
        nc.sync.dma_start(out=w_enc_sb[:], in_=w_enc_d[:])
        w_out_sb = const.tile([128, NJC * V], BF16)
        nc.sync.dma_start(out=w_out_sb[:], in_=w_out_d[:])
        bias_rep = const.tile([128, V], F32)
        nc.gpsimd.dma_start(
            out=bias_rep[:],
            in_=b_out_d[:].unsqueeze(0).broadcast_to((128, V)),
        )

        enc_projT = [const.tile([128, T], F32, name=f"ep{jc}") for jc in range(NJC)]
        dec_projT = [const.tile([128, U], F32, name=f"dp{jc}") for jc in range(NJC)]

        # --- setup: input projections (bf16 mms, fp32 accumulation) ---------
        with tc.tile_pool(name="setup_ps", bufs=4, space="PSUM") as setup_ps:
            for jc in range(NJC):
                ps = setup_ps.tile([128, U], F32, tag="dproj")
                for dc in range(NDC):
                    nc.tensor.matmul(
                        ps[:],
                        w_dec_sb[:, dc * J + jc * 128 : dc * J + (jc + 1) * 128],
                        decT[:, dc * U : (dc + 1) * U],
                        start=(dc == 0),
                        stop=(dc == NDC - 1),
                    )
                # fold b_enc+b_dec into dec_projT during the PSUM->SBUF drain
                nc.scalar.activation(
                    dec_projT[jc][:], ps[:], ACT_F.Identity,
                    bias=bbt[:, jc : jc + 1], scale=1.0,
                )

                ps = setup_ps.tile([128, T], F32, tag="eproj")
                for dc in range(NDC):
                    nc.tensor.matmul(
                        ps[:],
                        w_enc_sb[:, dc * J + jc * 128 : dc * J + (jc + 1) * 128],
                        encT[:, dc * T : (dc + 1) * T],
                        start=(dc == 0),
                        stop=(dc == NDC - 1),
                    )
                nc.vector.tensor_copy(enc_projT[jc][:], ps[:])

        # --- main loop over u-blocks ----------------------------------------
        h_pool = ctx.enter_context(tc.tile_pool(name="h", bufs=3))
        st_pool = ctx.enter_context(tc.tile_pool(name="stage", bufs=4))
        mm_ps = ctx.enter_context(tc.tile_pool(name="mm_ps", bufs=4, space="PSUM"))

        for ub in range(U // UB):
            u0 = ub * UB
            hT = [
                [
                    h_pool.tile([128, T], BF16, tag=f"h{jc}_{uu}", name=f"h{jc}_{uu}")
                    for uu in range(UB)
                ]
                for jc in range(NJC)
            ]
            for jc in range(NJC):
                for uu in range(UB):
                    nc.scalar.activation(
                        hT[jc][uu][:],
                        enc_projT[jc][:],
                        ACT_F.Tanh,
                        bias=dec_projT[jc][:, u0 + uu : u0 + uu + 1],
                        scale=1.0,
                    )
            for tt in range(T // 128):
                for uu in range(UB):
                    ps = mm_ps.tile([128, V], F32, tag="mm")
                    for vv in range(NVB):
                        for jc in range(NJC):
                            nc.tensor.matmul(
                                ps[:, vv * 512 : (vv + 1) * 512],
                                hT[jc][uu][:, tt * 128 : (tt + 1) * 128],
                                w_out_sb[:, jc * V + vv * 512 : jc * V + (vv + 1) * 512],
                                start=(jc == 0),
                                stop=(jc == NJC - 1),
                            )
                    # drain PSUM -> SBUF while adding the broadcast b_out
                    stage = st_pool.tile([128, 1, V], F32, tag="st")
                    nc.vector.tensor_add(stage[:, 0, :], ps[:], bias_rep[:])
                    nc.sync.dma_start(
                        out=out[tt * 128 : (tt + 1) * 128, u0 + uu : u0 + uu + 1, :],
                        in_=stage[:],
                    )

    nc.finalize()
    return nc


_PROGRAM = None


def _pack(a: np.ndarray, nchunk: int) -> np.ndarray:
    """[nchunk*128, W] -> [128, nchunk*W] with pk[p, c*W+x] = a[c*128+p, x]."""
    w = a.shape[1]
    return np.ascontiguousarray(
        a.reshape(nchunk, 128, w).transpose(1, 0, 2).reshape(128, nchunk * w)
    )


def _make_in_maps(enc_out, dec_out, W_enc, b_enc, W_dec, b_dec, W_out, b_out):
    import ml_dtypes

    bf16 = ml_dtypes.bfloat16
    bb = (np.asarray(b_enc, np.float32) + np.asarray(b_dec, np.float32))
    bb_pk = np.ascontiguousarray(bb.reshape(NJC, 128).T)
    w_enc_pk = _pack(np.asarray(W_enc, np.float32), NDC).astype(bf16)
    w_dec_pk = _pack(np.asarray(W_dec, np.float32), NDC).astype(bf16)
    w_out_pk = _pack(np.asarray(W_out, np.float32), NJC).astype(bf16)
    b_out_f = np.asarray(b_out, np.float32)
    enc_f = np.asarray(enc_out, np.float32)
    dec_f = np.asarray(dec_out, np.float32)

    in_maps = []
    for b in range(B):
        in_maps.append(
            {
                "encT": _pack(np.ascontiguousarray(enc_f[b, :, 0, :].T), NDC).astype(bf16),
                "decT": _pack(np.ascontiguousarray(dec_f[b, 0, :, :].T), NDC).astype(bf16),
                "w_enc": w_enc_pk,
                "w_dec": w_dec_pk,
                "bb": bb_pk,
                "w_out": w_out_pk,
                "b_out": b_out_f,
            }
        )
    return in_maps


def kernel(enc_out, dec_out, W_enc, b_enc, W_dec, b_dec, W_out, b_out):
    global _PROGRAM
    if _PROGRAM is None:
        _PROGRAM = build_program()

    in_maps = _make_in_maps(
        enc_out, dec_out, W_enc, b_enc, W_dec, b_dec, W_out, b_out
    )
    res = run_bass_kernel_spmd(_PROGRAM, in_maps, list(range(B)))
    return np.stack([res.results[b]["out"] for b in range(B)], axis=0)
